# revision 26
# baseline (speedup 1.0000x reference)
"""Multi-head attention TRN2 kernel.

Sharding: 8 cores = 4 batches x 2 head-groups (Megatron tensor parallel over
the 16 heads: Wq/Wk/Wv column-sharded, Wo row-sharded; partial outputs summed
per batch on the host).

Per-core device kernel (batch b, head-group hg -> heads 8hg..8hg+8):
  v  = xvT.T  @ WvT           [2048, 512]  + ones column per head
  qT = WqT.T @ xqT            [512, 2048]  (d-major: heads pairwise stacked)
  kT = WkT.T @ xkT            [512, 2048]
  per (s_tile 512, head-pair, t_chunk 128):
     scoresT[t,s] = kT_h.T @ qT_h     (K=64, heads row-packed 0-63/64-127)
     exp: split between ACT (exact, scale folded) and DVE (Schraudolph int16
     bit-trick straight into bf16 bits) to relieve the scalar engine
     ctxT_aug[65,s] += v_aug.T @ expT (ones row accumulates softmax sums)
  normalize: PSUM-freeing copies first, then approx-reciprocal (DVE custom op)
  + gpsimd broadcast/mul off the critical path -> ctxT [512,2048]
  out_partial = ctxT.T @ WoT  [2048, 1024]  (interleaved per head-pair)
"""

import os
import sys
from contextlib import ExitStack

for _p in ("/opt/trn_rl_repo", "/root/.axon_site/_ro/trn_rl_repo"):
    if os.path.isdir(_p) and _p not in sys.path:
        sys.path.insert(0, _p)
        break

import numpy as np

import concourse.bass as bass
import concourse.bacc as bacc
import concourse.mybir as mybir
import concourse.tile as tile

B, S, E, H, D = 4, 2048, 1024, 16, 64
HG = 2          # head groups (tensor-parallel factor)
DH = E // HG    # 512 dims per head group (8 heads)
HPG = H // HG   # 8 heads per group
NCORES = B * HG

F32 = mybir.dt.float32
I16 = mybir.dt.int16
_MM_DT_NAME = os.environ.get("BASS_MHA_MM_DT", "bfloat16")
MM_DT = getattr(mybir.dt, _MM_DT_NAME)

SCALE = 1.0 / np.sqrt(D)

# Schraudolph exp in bf16-bit space: i16 = round(x*EXP_A + EXP_B) are the bf16
# bits of ~exp(x*SCALE).  EXP_A = 128*log2(e)*SCALE; EXP_B tuned numerically
# for minimax relative error (~3.3%).
EXP_A = 128.0 / np.log(2.0) * SCALE
EXP_B = 16250.5
# which t-chunks (of 16) run their exp on the DVE instead of ACT
_DVE_T = os.environ.get("BASS_MHA_DVE_T", "2,5,8,11,14")
DVE_T = frozenset(int(t) for t in _DVE_T.split(",") if t != "")


def _mm(nc, out, lhsT, rhs, start, stop):
    if lhsT.dtype != MM_DT:
        lhsT = lhsT.bitcast(MM_DT)
    if rhs.dtype != MM_DT:
        rhs = rhs.bitcast(MM_DT)
    nc.tensor.matmul(out, lhsT=lhsT, rhs=rhs, start=start, stop=stop)


def build_nc():
    nc = bacc.Bacc()
    xqT = nc.declare_dram_parameter("xqT", [E, S], MM_DT, isOutput=False)
    xkT = nc.declare_dram_parameter("xkT", [E, S], MM_DT, isOutput=False)
    xvT = nc.declare_dram_parameter("xvT", [E, S], MM_DT, isOutput=False)
    wqT = nc.declare_dram_parameter("wqT", [E, DH], MM_DT, isOutput=False)
    wkT = nc.declare_dram_parameter("wkT", [E, DH], MM_DT, isOutput=False)
    wvT = nc.declare_dram_parameter("wvT", [E, DH], MM_DT, isOutput=False)
    woT = nc.declare_dram_parameter("woT", [DH, E], MM_DT, isOutput=False)
    out = nc.declare_dram_parameter("out", [S, E], F32, isOutput=True)

    with (
        nc.allow_low_precision(reason="bf16 matmul operands + approx exp"),
        tile.TileContext(nc) as tc,
        ExitStack() as ctx,
    ):
        _emit(ctx, tc, xqT, xkT, xvT, wqT, wkT, wvT, woT, out)
    nc.compile()
    return nc


def _norm_finish(nc, small, ctxT_sb, st, c, cxs):
    """Deferred tail of the softmax normalization for head pair (st, c)."""
    s0 = 512 * st
    for j in range(2):
        sums_st = small.tile([1, 512], mybir.dt.float32, tag="sums_st")
        nc.vector.tensor_copy(sums_st, cxs[j][D : D + 1, :])
        rec = small.tile([1, 512], mybir.dt.float32, tag="rec")
        nc.vector.reciprocal_approx_fast(out=rec, in_=sums_st)
        bc_sb = small.tile([64, 512], mybir.dt.float32, tag="bcsb")
        nc.gpsimd.partition_broadcast(bc_sb, rec)
        nc.vector.tensor_mul(
            ctxT_sb[64 * j : 64 * (j + 1), c, s0 : s0 + 512],
            cxs[j][0:D, :],
            bc_sb,
        )


def _proj_chunk(nc, dps, osb, ctxT_sb, wo_sb, out, s0, si):
    """Output projection for rows [s0+128*si, s0+128*si+128).  Borrows one
    rotation slot of the score-PSUM tag (no dedicated PSUM pool)."""
    DC = DH // 128
    r0 = s0 + 128 * si
    o_sb = osb.tile([128, E], F32, tag="osb", name=f"osb_{r0}")
    fp = dps.tile([128, 1024], F32, tag="sc", name=f"fp_{r0}")
    for et in range(2):
        for c in range(DC):
            _mm(
                nc,
                fp[:, 512 * et : 512 * (et + 1)],
                ctxT_sb[:, c, r0 : r0 + 128],
                wo_sb[:, c, 512 * et : 512 * (et + 1)],
                start=(c == 0),
                stop=(c == DC - 1),
            )
    for et in range(2):
        nc.vector.tensor_copy(
            o_sb[:, 512 * et : 512 * (et + 1)], fp[:, 512 * et : 512 * (et + 1)]
        )
        nc.sync.dma_start(
            out=out[r0 : r0 + 128, 512 * et : 512 * (et + 1)],
            in_=o_sb[:, 512 * et : 512 * (et + 1)],
        )


def _emit(ctx, tc, xqT, xkT, xvT, wqT, wkT, wvT, woT, out):
    nc = tc.nc
    EC = E // 128    # 8 contraction chunks for projections
    DC = DH // 128   # 4 d-chunks of the head group
    TC = S // 128    # 16 t chunks
    ST = S // 512    # 4 s tiles
    DA = D + 1       # 65: head dim + ones column
    QS = (nc.sync, nc.scalar, nc.gpsimd)   # DMA queue round-robin

    # ---- persistent tensors ----------------------------------------------
    big = ctx.enter_context(tc.tile_pool(name="big", bufs=1))
    qT_sb = big.tile([128, DC, S], MM_DT, tag="qT")
    kT_sb = big.tile([128, DC, S], MM_DT, tag="kT")
    v_sb = big.tile([128, TC, HPG, DA], MM_DT, tag="v")
    ctxT_sb = big.tile([128, DC, S], MM_DT, tag="ctx")
    ones_col = big.tile([128, 1], F32, tag="ones_col")
    nc.vector.memset(ones_col, 1.0)
    wo_sb = big.tile([128, DC, E], MM_DT, tag="wo")
    # prewarm the ACT exp table set while DMAs stream
    warm = big.tile([1, 8], F32, tag="warm")
    nc.vector.memset(warm, 0.0)
    nc.scalar.activation(
        out=warm, in_=warm, func=mybir.ActivationFunctionType.Exp, scale=1.0
    )
    for t in range(TC):
        nc.vector.tensor_copy(
            v_sb[:, t, :, D : D + 1],
            ones_col.to_broadcast((128, HPG)).rearrange("p (h o) -> p h o", o=1),
        )

    with (
        tc.tile_pool(name="wvp", bufs=1) as wvp,
        tc.tile_pool(name="xvin", bufs=8) as xvin,
        tc.tile_pool(name="wqk", bufs=1) as wqk,
        tc.tile_pool(name="xin", bufs=10) as xin,
    ):
        # wv first: it gates the very first matmul of stage C
        wv_sb = wvp.tile([128, EC, DH], MM_DT, tag="wv")
        wq_sb = wqk.tile([128, EC, DH], MM_DT, tag="wq")
        wk_sb = wqk.tile([128, EC, DH], MM_DT, tag="wk")
        for e in range(EC):
            QS[e % 3].dma_start(
                out=wv_sb[:, e, :], in_=wvT[128 * e : 128 * (e + 1), :]
            )

        # ---- stage C: v projection -----------------------------------
        # v[t, d] accumulated over e: lhsT = xvT block [e, t], rhs = WvT [e, d]
        with tc.tile_pool(name="cps", bufs=1, space="PSUM") as cps:
            for ph in range(4):  # 4 t-chunks per phase
                pv = []
                for tt in range(4):
                    pv.append(
                        cps.tile([128, DH], F32, tag=f"pv{tt}", bufs=2, name=f"pv{ph}_{tt}")
                    )
                for e in range(EC):
                    xtr = xvin.tile([128, 512], MM_DT, tag="xvt")
                    QS[e % 3].dma_start(
                        out=xtr,
                        in_=xvT[128 * e : 128 * (e + 1), 512 * ph : 512 * (ph + 1)],
                    )
                    for tt in range(4):
                        _mm(
                            nc,
                            pv[tt],
                            xtr[:, 128 * tt : 128 * (tt + 1)],
                            wv_sb[:, e, :],
                            start=(e == 0),
                            stop=(e == EC - 1),
                        )
                for tt in range(4):
                    t = 4 * ph + tt
                    nc.vector.tensor_copy(
                        v_sb[:, t, :, 0:D],
                        pv[tt].rearrange("p (h d) -> p h d", h=HPG),
                    )

        # q/k weights after the xv tiles they'd otherwise delay
        for e in range(EC):
            QS[e % 3].dma_start(
                out=wq_sb[:, e, :], in_=wqT[128 * e : 128 * (e + 1), :]
            )
            QS[(e + 1) % 3].dma_start(
                out=wk_sb[:, e, :], in_=wkT[128 * e : 128 * (e + 1), :]
            )

        # ---- stage B: qT / kT projections (q first; k last feeds D) ---
        with tc.tile_pool(name="bps", bufs=1, space="PSUM") as bps:
            for x_dram, w_sb, dst in ((xqT, wq_sb, qT_sb), (xkT, wk_sb, kT_sb)):
                for sh in range(2):  # s halves of 1024
                    xts = []
                    for e in range(EC):
                        xtr = xin.tile([128, 1024], MM_DT, tag="xt")
                        QS[e % 3].dma_start(
                            out=xtr,
                            in_=x_dram[
                                128 * e : 128 * (e + 1), 1024 * sh : 1024 * (sh + 1)
                            ],
                        )
                        xts.append(xtr)
                    for sq in range(2):
                        ps = []
                        for dc in range(DC):
                            ps.append(
                                bps.tile(
                                    [128, 512], F32, tag=f"pb{dc}", bufs=2,
                                    name=f"pb{dc}_{sh}_{sq}",
                                )
                            )
                        for e in range(EC):
                            for dc in range(DC):
                                _mm(
                                    nc,
                                    ps[dc],
                                    w_sb[:, e, 128 * dc : 128 * (dc + 1)],
                                    xts[e][:, 512 * sq : 512 * (sq + 1)],
                                    start=(e == 0),
                                    stop=(e == EC - 1),
                                )
                        for dc in range(DC):
                            nc.vector.tensor_copy(
                                dst[:, dc, 1024 * sh + 512 * sq : 1024 * sh + 512 * (sq + 1)],
                                ps[dc],
                            )

        # wo last: first needed by the output projection (one s-tile in)
        for a in range(DC):
            QS[a % 3].dma_start(
                out=wo_sb[:, a, :], in_=woT[128 * a : 128 * (a + 1), :]
            )

    # ---- stage D/E: attention + output projection ------------------------
    with (
        tc.tile_pool(name="ex", bufs=5) as expool,
        tc.tile_pool(name="small", bufs=3) as small,
        tc.tile_pool(name="osb", bufs=3) as osb,
        tc.tile_pool(name="dps", bufs=3, space="PSUM") as dps,
        tc.tile_pool(name="cxps", bufs=2, space="PSUM") as cxps,
    ):
        pending = []
        for st in range(ST):
            s0 = 512 * st
            for c in range(DC):  # head pair (2c, 2c+1)
                cx = [cxps.tile([DA, 512], F32, tag="cx", name=f"cx{st}_{c}_{j2}") for j2 in range(2)]
                # software pipeline: scores/exp for t are emitted BEFORE
                # ctx for t-1, so the PE FIFO never head-of-line blocks on
                # the exp result.
                exs = [None] * TC
                for t in range(TC + 2):
                    if t < TC:
                        sc = dps.tile([128, 1024], F32, tag="sc")
                        for j in range(2):
                            _mm(
                                nc,
                                sc[:, 512 * j : 512 * (j + 1)],
                                kT_sb[64 * j : 64 * (j + 1), c, 128 * t : 128 * (t + 1)],
                                qT_sb[64 * j : 64 * (j + 1), c, s0 : s0 + 512],
                                start=True,
                                stop=True,
                            )
                        ex = expool.tile([128, 1024], MM_DT, tag="ex")
                        exs[t] = ex
                        if t in DVE_T:
                            nc.vector.tensor_scalar(
                                out=ex[:, :].bitcast(I16),
                                in0=sc[:, :],
                                scalar1=float(EXP_A),
                                scalar2=float(EXP_B),
                                op0=mybir.AluOpType.mult,
                                op1=mybir.AluOpType.add,
                            )
                        else:
                            nc.scalar.activation(
                                out=ex,
                                in_=sc,
                                func=mybir.ActivationFunctionType.Exp,
                                scale=float(SCALE),
                            )
                    if t >= 2:
                        tp = t - 2
                        for j in range(2):
                            _mm(
                                nc,
                                cx[j],
                                v_sb[:, tp, 2 * c + j, :],
                                exs[tp][:, 512 * j : 512 * (j + 1)],
                                start=(tp == 0),
                                stop=(tp == TC - 1),
                            )
                # free cx psum slots fast: both PSUM->SBUF copies come first.
                # The rest of the norm chain for this head pair is DEFERRED
                # by one c-iteration so it never delays the next t-loop's
                # exps in the DVE FIFO.
                cxs = []
                for j in range(2):
                    cxs.append(small.tile([DA, 512], F32, tag="cxs", bufs=6, name=f"cxs{st}_{c}_{j}"))
                    nc.vector.tensor_copy(cxs[j], cx[j])
                for pst, pc, pcxs in pending:
                    _norm_finish(nc, small, ctxT_sb, pst, pc, pcxs)
                pending = [(st, c, cxs)]
                # interleave one output-projection chunk of the PREVIOUS
                # s-tile after each head pair (hides the norm chain)
                if st > 0:
                    _proj_chunk(nc, dps, osb, ctxT_sb, wo_sb, out, 512 * (st - 1), c)
        for pst, pc, pcxs in pending:
            _norm_finish(nc, small, ctxT_sb, pst, pc, pcxs)
        for si in range(4):
            _proj_chunk(nc, dps, osb, ctxT_sb, wo_sb, out, 512 * (ST - 1), si)


_BUILT = {}


def _get_nc():
    if "nc" not in _BUILT:
        _BUILT["nc"] = build_nc()
    return _BUILT["nc"]


def make_in_maps(query, key, value, Wq, Wk, Wv, Wo):
    ndt = mybir.dt.np(MM_DT)
    query = np.asarray(query, np.float32).astype(ndt)
    key = np.asarray(key, np.float32).astype(ndt)
    value = np.asarray(value, np.float32).astype(ndt)
    Wq = np.asarray(Wq, np.float32).astype(ndt)
    Wk = np.asarray(Wk, np.float32).astype(ndt)
    Wv = np.asarray(Wv, np.float32).astype(ndt)
    Wo = np.asarray(Wo, np.float32).astype(ndt)

    xqT = [np.ascontiguousarray(query[b].T) for b in range(B)]
    xkT = [np.ascontiguousarray(key[b].T) for b in range(B)]
    xvT = [np.ascontiguousarray(value[b].T) for b in range(B)]
    wqT = [np.ascontiguousarray(Wq[DH * g : DH * (g + 1), :].T) for g in range(HG)]
    wkT = [np.ascontiguousarray(Wk[DH * g : DH * (g + 1), :].T) for g in range(HG)]
    wvT = [np.ascontiguousarray(Wv[DH * g : DH * (g + 1), :].T) for g in range(HG)]
    woT = [np.ascontiguousarray(Wo[:, DH * g : DH * (g + 1)].T) for g in range(HG)]

    in_maps = []
    for core in range(NCORES):
        b, g = core // HG, core % HG
        in_maps.append(
            {
                "xqT": xqT[b],
                "xkT": xkT[b],
                "xvT": xvT[b],
                "wqT": wqT[g],
                "wkT": wkT[g],
                "wvT": wvT[g],
                "woT": woT[g],
            }
        )
    return in_maps


def assemble(core_outs):
    out = np.empty((B, S, E), np.float32)
    for b in range(B):
        out[b] = core_outs[HG * b]
        for g in range(1, HG):
            out[b] += core_outs[HG * b + g]
    return out


def kernel(query, key, value, Wq, Wk, Wv, Wo):
    from concourse.bass_utils import run_bass_kernel_spmd

    nc = _get_nc()
    in_maps = make_in_maps(query, key, value, Wq, Wk, Wv, Wo)
    res = run_bass_kernel_spmd(nc, in_maps, list(range(NCORES)))
    return assemble([r["out"] for r in res.results])
